# revision 3
# baseline (speedup 1.0000x reference)
"""Trainium2 Bass kernel for nn_CompressModel: y = FHT_1024(x * golay) / (alpha + eps).

Factorization: H_1024 = H_8 (outer, feat bits 7-9) (x) H_128 (inner, feat bits 0-6).
H_128 is applied FIRST (it commutes with H_8), which lets the golay signs fold into
8 pre-signed matrices Ha = diag(golay[a*128:(a+1)*128]) @ H_128 (entries +-1, exact
in bf16). The 1/(alpha+eps) scale rides the PSUM drain as an ACT scale vector.

Internally everything is bf16 (rel-err gate is 2e-2; bf16 keeps us ~4e-3):
  - PE transposes and matmuls run single-pass (f32 would be LOW_HIGH split)
  - vector butterflies hit the 2x_1P perf mode
  - f32<->bf16 conversion is free: SWDGE (gpsimd) DMA casts in flight both directions

Per-core dataflow (4096 rows/core, supertiles of 256 rows = 2 row-groups):
  1. gpsimd DMA x supertile [256, 1024] f32 -> SBUF bf16 [128p, 2rg x 1024f]
  2. PE transposes 16 [128r, 128b] blocks -> PSUM zt (bf16, 2 banks)
  3. one ScalarE drain zt -> SBUF t (lhsT blocks [b, r]), FD=2048
  4. 16 PE matmuls po[r, a*128+b'] = t_a^T @ Ha  (golay folded in) -> PSUM f32
  5. one ScalarE drain po -> SBUF bf16 with 1/(alpha+eps) scale, FD=2048
  6. VectorE: outer FHT_8 = 3 butterfly levels across a-blocks (bf16, 2x mode)
  7. gpsimd DMA out: SBUF bf16 -> HBM f32 (cast in flight)
"""

import numpy as np
from contextlib import ExitStack

import ml_dtypes

import concourse.bass as bass
import concourse.tile as tile
from concourse import bacc, mybir
from concourse.bass_utils import run_bass_kernel_spmd

f32 = mybir.dt.float32
bf16 = mybir.dt.bfloat16

N_CORES = 8
DIM = 1024
EPS = 1e-5
ROWS_TOTAL = 4 * 8192          # 32768
ROWS_PER_CORE = ROWS_TOTAL // N_CORES   # 4096
ST_ROWS = 256                  # rows per supertile
N_ST = ROWS_PER_CORE // ST_ROWS  # 16
RG = ST_ROWS // 128            # row-groups per supertile (2)

LAST_RESULT = None  # test harness reads exec_time_ns from here


def _hadamard(n: int) -> np.ndarray:
    h = np.array([[1.0]], dtype=np.float32)
    while h.shape[0] < n:
        h = np.block([[h, h], [h, -h]])
    return np.ascontiguousarray(h.astype(np.float32))


def _golay(n: int) -> np.ndarray:
    a = np.array([1.0], dtype=np.float32)
    b = np.array([1.0], dtype=np.float32)
    k = 1
    while k < n:
        a, b = np.concatenate([a, b]), np.concatenate([a, -b])
        k *= 2
    return a


def _build_nc():
    nc = bacc.Bacc("TRN2", target_bir_lowering=False, debug=False)
    x_d = nc.dram_tensor("x", [ROWS_PER_CORE, DIM], f32, kind="ExternalInput")
    ha_d = nc.dram_tensor("ha", [128, 8 * 128], bf16, kind="ExternalInput")
    i_d = nc.dram_tensor("identb", [128, 128], bf16, kind="ExternalInput")
    s_d = nc.dram_tensor("svec", [128, 1], f32, kind="ExternalInput")
    y_d = nc.dram_tensor("y", [ROWS_PER_CORE, DIM], f32, kind="ExternalOutput")

    with TileKernel(nc) as tk:
        tk.emit(x_d, ha_d, i_d, s_d, y_d)

    nc.compile()
    return nc


class TileKernel:
    def __init__(self, nc):
        self.nc = nc
        self.ctx = ExitStack()

    def __enter__(self):
        self.tc = self.ctx.enter_context(tile.TileContext(self.nc))
        return self

    def __exit__(self, *exc):
        return self.ctx.__exit__(*exc)

    def emit(self, x_d, ha_d, i_d, s_d, y_d):
        nc, tc, ctx = self.nc, self.tc, self.ctx

        const_pool = ctx.enter_context(tc.tile_pool(name="const", bufs=1))
        x_pool = ctx.enter_context(tc.tile_pool(name="x", bufs=5))
        t_pool = ctx.enter_context(tc.tile_pool(name="t", bufs=3))
        y0_pool = ctx.enter_context(tc.tile_pool(name="y0", bufs=2))
        v1_pool = ctx.enter_context(tc.tile_pool(name="v1", bufs=2))
        v2_pool = ctx.enter_context(tc.tile_pool(name="v2", bufs=2))
        v3_pool = ctx.enter_context(tc.tile_pool(name="v3", bufs=3))
        zt_pool = ctx.enter_context(tc.tile_pool(name="zt", bufs=2, space="PSUM"))
        po_pool = ctx.enter_context(tc.tile_pool(name="po", bufs=1, space="PSUM"))

        identb = const_pool.tile([128, 128], bf16)
        nc.sync.dma_start(identb[:], i_d.ap()[:, :])
        ha = const_pool.tile([128, 8 * 128], bf16)
        nc.sync.dma_start(ha[:], ha_d.ap()[:, :])
        svec = const_pool.tile([128, 1], f32)
        nc.sync.dma_start(svec[:], s_d.ap()[:, :])

        W = RG * DIM  # 2048

        for st in range(N_ST):
            r0 = st * ST_ROWS
            # ---- load supertile, casting f32 -> bf16 in flight (SWDGE) ----
            x_st = x_pool.tile([128, W], bf16)
            nc.gpsimd.dma_start(
                x_st[:].rearrange("p (rg f) -> p rg f", rg=RG),
                x_d.ap()[r0:r0 + ST_ROWS, :].rearrange("(rg p) f -> p rg f", p=128),
            )

            # ---- transpose-in: zt[b, rg*1024 + a*128 + r] = x[rg, r, a*128+b] ----
            zt = zt_pool.tile([128, W], bf16)
            for rg in range(RG):
                for a in range(8):
                    o = rg * DIM + a * 128
                    nc.tensor.transpose(
                        zt[:, o:o + 128], x_st[:, o:o + 128], identb[:],
                    )
            t = t_pool.tile([128, W], bf16)
            nc.scalar.copy(t[:], zt[:])

            # ---- inner H_128 (golay-signed) matmuls, un-transpose ----
            po = po_pool.tile([128, W], f32)
            for rg in range(RG):
                for a in range(8):
                    o = rg * DIM + a * 128
                    nc.tensor.matmul(
                        po[:, o:o + 128],
                        lhsT=t[:, o:o + 128],
                        rhs=ha[:, a * 128:(a + 1) * 128],
                        start=True, stop=True,
                    )
            # drain + 1/(alpha+eps) scale, f32 -> bf16
            y0 = y0_pool.tile([128, W], bf16)
            nc.scalar.mul(y0[:], po[:], svec[:, 0:1])

            # ---- outer FHT_8: 3 butterfly levels over a-blocks (bf16, 2x) ----
            v1 = v1_pool.tile([128, W], bf16)
            y0r = y0[:].rearrange("p (rg a c) -> p rg a c", rg=RG, a=8)
            v1r = v1[:].rearrange("p (rg a c) -> p rg a c", rg=RG, a=8)
            nc.vector.tensor_add(v1r[:, :, 0:4, :], y0r[:, :, 0:4, :], y0r[:, :, 4:8, :])
            nc.vector.tensor_sub(v1r[:, :, 4:8, :], y0r[:, :, 0:4, :], y0r[:, :, 4:8, :])

            v2 = v2_pool.tile([128, W], bf16)
            v1q = v1[:].rearrange("p (g a2 dc) -> p g a2 dc", g=2 * RG, a2=2)
            v2q = v2[:].rearrange("p (g a2 dc) -> p g a2 dc", g=2 * RG, a2=2)
            nc.vector.tensor_add(v2q[:, :, 0:1, :], v1q[:, :, 0:1, :], v1q[:, :, 1:2, :])
            nc.vector.tensor_sub(v2q[:, :, 1:2, :], v1q[:, :, 0:1, :], v1q[:, :, 1:2, :])

            v3 = v3_pool.tile([128, W], bf16)
            v2h = v2[:].rearrange("p (h a3 c) -> p h a3 c", h=4 * RG, a3=2)
            v3h = v3[:].rearrange("p (h a3 c) -> p h a3 c", h=4 * RG, a3=2)
            nc.vector.tensor_add(v3h[:, :, 0:1, :], v2h[:, :, 0:1, :], v2h[:, :, 1:2, :])
            nc.vector.tensor_sub(v3h[:, :, 1:2, :], v2h[:, :, 0:1, :], v2h[:, :, 1:2, :])

            # ---- store, casting bf16 -> f32 in flight (SWDGE) ----
            nc.gpsimd.dma_start(
                y_d.ap()[r0:r0 + ST_ROWS, :].rearrange("(rg p) f -> p rg f", p=128),
                v3[:].rearrange("p (rg f) -> p rg f", rg=RG),
            )


_NC = None


def _get_nc():
    global _NC
    if _NC is None:
        _NC = _build_nc()
    return _NC


def kernel(x, golay, alpha):
    global LAST_RESULT
    x_np = np.ascontiguousarray(np.asarray(x, dtype=np.float32).reshape(ROWS_TOTAL, DIM))
    golay_np = np.asarray(golay, dtype=np.float32).reshape(DIM)
    alpha_np = np.float32(np.asarray(alpha, dtype=np.float32))

    s = np.float32(1.0) / (alpha_np + np.float32(EPS))
    hmat = _hadamard(128)
    # Ha[b, a*128+b'] = golay[a*128+b] * H128[b, b']  (entries +-1, exact in bf16)
    ha_np = np.empty((128, 8 * 128), dtype=np.float32)
    for a in range(8):
        ha_np[:, a * 128:(a + 1) * 128] = golay_np[a * 128:(a + 1) * 128, None] * hmat
    ha_np = ha_np.astype(ml_dtypes.bfloat16)
    ident_np = np.eye(128, dtype=np.float32).astype(ml_dtypes.bfloat16)
    svec_np = np.full((128, 1), s, dtype=np.float32)

    nc = _get_nc()
    in_maps = [
        {
            "x": x_np[c * ROWS_PER_CORE:(c + 1) * ROWS_PER_CORE],
            "ha": ha_np,
            "identb": ident_np,
            "svec": svec_np,
        }
        for c in range(N_CORES)
    ]
    res = run_bass_kernel_spmd(nc, in_maps, core_ids=list(range(N_CORES)))
    LAST_RESULT = res
    y = np.concatenate([r["y"] for r in res.results], axis=0)
    return y.reshape(4, 8192, DIM)


# revision 4
# speedup vs baseline: 1.0862x; 1.0862x over previous
"""Trainium2 Bass kernel for nn_CompressModel: y = FHT_1024(x * golay) / (alpha + eps).

Factorization: H_1024 = H_2 (feat bit 9) (x) H_4 (feat bits 7-8) (x) H_128 (bits 0-6).
H_128 is applied first (all levels commute), golay signs fold into 8 pre-signed
matrices Ha = diag(golay_block_a) @ H_128 (entries +-1, exact in bf16). The H_2
level is folded into the PE via paired accumulating matmuls with +-Ha, so the
vector engine only runs the 2 remaining H_4 butterfly levels. The 1/(alpha+eps)
scale rides the PSUM drain as an ACT scale vector.

Internally everything is bf16 (rel-err gate is 2e-2; bf16 keeps us ~4e-3).
f32<->bf16 conversion is free: SWDGE (gpsimd) DMA casts in flight both directions.

Per-core dataflow (4096 rows/core, supertiles of 256 rows = 2 row-groups):
  1. gpsimd DMA x supertile [256, 1024] f32 -> SBUF bf16 [128p, 2rg x 1024f]
  2. per rg: PE transposes 8 [128r, 128b] blocks -> PSUM zt (bf16, 1 bank)
  3. VectorE drains zt -> SBUF t (lhsT blocks [b, r]); bf16 PSUM read packs 2x
  4. per rg: 16 paired PE matmuls po[r, a2*128+b'] = t_a^T Ha_a +- t_a'^T Ha_a'
     (H_128, golay, and the H_2 butterfly all in one PSUM accumulation)
  5. ScalarE drains po -> SBUF bf16 with 1/(alpha+eps) scale
  6. VectorE: 2 H_4 butterfly levels across a2-blocks (bf16, 2x mode)
  7. gpsimd DMA out: SBUF bf16 -> HBM f32 (cast in flight)
"""

import numpy as np
from contextlib import ExitStack

import ml_dtypes

import concourse.bass as bass
import concourse.tile as tile
from concourse import bacc, mybir
from concourse.bass_utils import run_bass_kernel_spmd

f32 = mybir.dt.float32
bf16 = mybir.dt.bfloat16

N_CORES = 8
DIM = 1024
EPS = 1e-5
ROWS_TOTAL = 4 * 8192          # 32768
ROWS_PER_CORE = ROWS_TOTAL // N_CORES   # 4096
ST_ROWS = 256                  # rows per supertile
N_ST = ROWS_PER_CORE // ST_ROWS  # 16
RG = ST_ROWS // 128            # row-groups per supertile (2)

LAST_RESULT = None  # test harness reads exec_time_ns from here


def _hadamard(n: int) -> np.ndarray:
    h = np.array([[1.0]], dtype=np.float32)
    while h.shape[0] < n:
        h = np.block([[h, h], [h, -h]])
    return np.ascontiguousarray(h.astype(np.float32))


def _build_nc():
    nc = bacc.Bacc("TRN2", target_bir_lowering=False, debug=False)
    x_d = nc.dram_tensor("x", [ROWS_PER_CORE, DIM], f32, kind="ExternalInput")
    ha_d = nc.dram_tensor("ha", [128, 8 * 128], bf16, kind="ExternalInput")
    han_d = nc.dram_tensor("han", [128, 8 * 128], bf16, kind="ExternalInput")
    i_d = nc.dram_tensor("identb", [128, 128], bf16, kind="ExternalInput")
    s_d = nc.dram_tensor("svec", [128, 1], f32, kind="ExternalInput")
    y_d = nc.dram_tensor("y", [ROWS_PER_CORE, DIM], f32, kind="ExternalOutput")

    with TileKernel(nc) as tk:
        tk.emit(x_d, ha_d, han_d, i_d, s_d, y_d)

    nc.compile()
    return nc


class TileKernel:
    def __init__(self, nc):
        self.nc = nc
        self.ctx = ExitStack()

    def __enter__(self):
        self.tc = self.ctx.enter_context(tile.TileContext(self.nc))
        return self

    def __exit__(self, *exc):
        return self.ctx.__exit__(*exc)

    def emit(self, x_d, ha_d, han_d, i_d, s_d, y_d):
        nc, tc, ctx = self.nc, self.tc, self.ctx

        const_pool = ctx.enter_context(tc.tile_pool(name="const", bufs=1))
        x_pool = ctx.enter_context(tc.tile_pool(name="x", bufs=5))
        t_pool = ctx.enter_context(tc.tile_pool(name="t", bufs=4))
        y0_pool = ctx.enter_context(tc.tile_pool(name="y0", bufs=2))
        v2_pool = ctx.enter_context(tc.tile_pool(name="v2", bufs=2))
        v3_pool = ctx.enter_context(tc.tile_pool(name="v3", bufs=3))
        zt_pool = ctx.enter_context(tc.tile_pool(name="zt", bufs=4, space="PSUM"))
        po_pool = ctx.enter_context(tc.tile_pool(name="po", bufs=2, space="PSUM"))

        # First input supertile goes before the consts in the gpsimd queue.
        x_first = x_pool.tile([128, RG * DIM], bf16)
        nc.gpsimd.dma_start(
            x_first[:].rearrange("p (rg f) -> p rg f", rg=RG),
            x_d.ap()[0:ST_ROWS, :].rearrange("(rg p) f -> p rg f", p=128),
        )

        identb = const_pool.tile([128, 128], bf16)
        nc.sync.dma_start(identb[:], i_d.ap()[:, :])
        ha = const_pool.tile([128, 8 * 128], bf16)
        nc.sync.dma_start(ha[:], ha_d.ap()[:, :])
        han = const_pool.tile([128, 8 * 128], bf16)
        nc.sync.dma_start(han[:], han_d.ap()[:, :])
        svec = const_pool.tile([128, 1], f32)
        nc.sync.dma_start(svec[:], s_d.ap()[:, :])

        for st in range(N_ST):
            r0 = st * ST_ROWS
            # ---- load supertile, casting f32 -> bf16 in flight (SWDGE) ----
            if st == 0:
                x_st = x_first
            else:
                x_st = x_pool.tile([128, RG * DIM], bf16)
                nc.gpsimd.dma_start(
                    x_st[:].rearrange("p (rg f) -> p rg f", rg=RG),
                    x_d.ap()[r0:r0 + ST_ROWS, :].rearrange("(rg p) f -> p rg f", p=128),
                )

            y0 = y0_pool.tile([128, RG * DIM], bf16)
            for rg in range(RG):
                f0 = rg * DIM
                # ---- transpose-in: zt[b, a*128+r] = x[r, a*128+b] ----
                zt = zt_pool.tile([128, DIM], bf16)
                for a in range(8):
                    nc.tensor.transpose(
                        zt[:, a * 128:(a + 1) * 128],
                        x_st[:, f0 + a * 128: f0 + (a + 1) * 128],
                        identb[:],
                    )
                t = t_pool.tile([128, DIM], bf16)
                nc.vector.tensor_copy(t[:], zt[:])

                # ---- H_128 (golay-signed) + H_2 butterfly, accumulated on PE ----
                po = po_pool.tile([128, DIM], f32)
                for a2 in range(8):
                    if a2 < 4:
                        # w[a2] = u[a2] + u[a2+4]
                        pairs = [(a2, ha), (a2 + 4, ha)]
                    else:
                        # w[a2] = u[a2-4] - u[a2]
                        pairs = [(a2 - 4, ha), (a2, han)]
                    for k, (a, hmat_t) in enumerate(pairs):
                        nc.tensor.matmul(
                            po[:, a2 * 128:(a2 + 1) * 128],
                            lhsT=t[:, a * 128:(a + 1) * 128],
                            rhs=hmat_t[:, a * 128:(a + 1) * 128],
                            start=(k == 0), stop=(k == 1),
                        )
                # drain + 1/(alpha+eps) scale, f32 -> bf16
                nc.scalar.mul(y0[:, f0:f0 + DIM], po[:], svec[:, 0:1])

            # ---- remaining H_4: 2 butterfly levels over a2-blocks (bf16, 2x) ----
            v2 = v2_pool.tile([128, RG * DIM], bf16)
            y0q = y0[:].rearrange("p (g a2 dc) -> p g a2 dc", g=2 * RG, a2=2)
            v2q = v2[:].rearrange("p (g a2 dc) -> p g a2 dc", g=2 * RG, a2=2)
            nc.vector.tensor_add(v2q[:, :, 0:1, :], y0q[:, :, 0:1, :], y0q[:, :, 1:2, :])
            nc.vector.tensor_sub(v2q[:, :, 1:2, :], y0q[:, :, 0:1, :], y0q[:, :, 1:2, :])

            v3 = v3_pool.tile([128, RG * DIM], bf16)
            v2h = v2[:].rearrange("p (h a3 c) -> p h a3 c", h=4 * RG, a3=2)
            v3h = v3[:].rearrange("p (h a3 c) -> p h a3 c", h=4 * RG, a3=2)
            nc.vector.tensor_add(v3h[:, :, 0:1, :], v2h[:, :, 0:1, :], v2h[:, :, 1:2, :])
            nc.vector.tensor_sub(v3h[:, :, 1:2, :], v2h[:, :, 0:1, :], v2h[:, :, 1:2, :])

            # ---- store, casting bf16 -> f32 in flight (SWDGE) ----
            nc.gpsimd.dma_start(
                y_d.ap()[r0:r0 + ST_ROWS, :].rearrange("(rg p) f -> p rg f", p=128),
                v3[:].rearrange("p (rg f) -> p rg f", rg=RG),
            )


_NC = None


def _get_nc():
    global _NC
    if _NC is None:
        _NC = _build_nc()
    return _NC


def kernel(x, golay, alpha):
    global LAST_RESULT
    x_np = np.ascontiguousarray(np.asarray(x, dtype=np.float32).reshape(ROWS_TOTAL, DIM))
    golay_np = np.asarray(golay, dtype=np.float32).reshape(DIM)
    alpha_np = np.float32(np.asarray(alpha, dtype=np.float32))

    s = np.float32(1.0) / (alpha_np + np.float32(EPS))
    hmat = _hadamard(128)
    # Ha[b, a*128+b'] = golay[a*128+b] * H128[b, b']  (entries +-1, exact in bf16)
    ha_np = np.empty((128, 8 * 128), dtype=np.float32)
    for a in range(8):
        ha_np[:, a * 128:(a + 1) * 128] = golay_np[a * 128:(a + 1) * 128, None] * hmat
    han_np = (-ha_np).astype(ml_dtypes.bfloat16)
    ha_np = ha_np.astype(ml_dtypes.bfloat16)
    ident_np = np.eye(128, dtype=np.float32).astype(ml_dtypes.bfloat16)
    svec_np = np.full((128, 1), s, dtype=np.float32)

    nc = _get_nc()
    in_maps = [
        {
            "x": x_np[c * ROWS_PER_CORE:(c + 1) * ROWS_PER_CORE],
            "ha": ha_np,
            "han": han_np,
            "identb": ident_np,
            "svec": svec_np,
        }
        for c in range(N_CORES)
    ]
    res = run_bass_kernel_spmd(nc, in_maps, core_ids=list(range(N_CORES)))
    LAST_RESULT = res
    y = np.concatenate([r["y"] for r in res.results], axis=0)
    return y.reshape(4, 8192, DIM)
